# revision 15
# baseline (speedup 1.0000x reference)
"""Child-Sum Tree-LSTM (complete binary tree, depth 15, heap layout) on
8 Trainium2 cores.

Sharding (per hint): data-parallel over nodes within each level.  The
heap layout makes per-level chunks self-contained: core j's chunk at
level l has its children exactly in core j's chunk at level l+1, so
levels 14..3 run with zero communication.  One AllGather ships the 8
level-3 (h,c) pairs to every core; levels 2..0 (7 nodes) are computed
replicated.

Compute recipe (dtype mix validated by numpy emulation, ~6e-3 rel err
vs the 2e-2 gate):
  - Levels 14..11 ("big", >=256 nodes/core): all matmuls fp8e4m3 with
    DoubleRow perf mode (K=256 per MM, ~2x effective rate over
    bf16/fp32r: measured 164-189 ns vs 202 ns per 512-wide MM at half
    the MM count).  Operand scaling x*16, h*16, W*64; psum dequant
    1/1024 via ACT scale.  h of levels 14..12 is stored fp8*16 only
    (consumed only by the next level's matmuls); c stays fp32; gates
    come out of ACT in fp16 so the DVE elementwise runs in 2x SBUF
    mode.  Leaf halves interleave with level-13 halves (ping-pong c/h8
    buffers); leaf stationaries are reused across two 512-col subtiles.
  - Levels 10..0 ("small", <=128 nodes/core): latency-bound, so their
    x-parts (+bias) for all 262 small+top nodes are precomputed in one
    batched fp16 pass (gx); each level then runs only the recurrent
    h-part matmuls in fp16 (48 for i/o/u + 16 for f with left/right
    children interleaved in the moving operand).  h buffers are fp16
    (the f-matmuls stream child h directly, no cast copy); forget-gate
    psums ping-pong between two banks so consecutive mo-chunks do not
    serialize.
  - Level 11 bridges the two: fp8 inputs, fp16 h output.

  - Forget-gate psums live in one paired 2-bank tile so fl/fr activate
    in a single ACT op; leaf tanh(c) and the h8 write are batched across
    the two 512-col subtiles (the big phase is ACT-limited, so fewer,
    larger ACT ops matter).  Within a small level the f matmuls issue
    before the i/o/u ones (they read child h directly, no wait on the
    child-sum), and the i/o gates pair into one tile for a single
    sigmoid ACT.

Timing on this host (serialized repeat deltas, async-pipelined calls —
wall-clock per call is ~100 ms of tunnel overhead that completely hides
device time, so only in-NEFF repeat deltas are meaningful): full tree
~171 us/core vs ~270-283 us for the staged fp32r baseline (~1.6x).
Attribution by stop_after ablation: big levels ~70 us (PE/ACT
balanced), small levels ~45 us, AllGather collective ~36 us (fixed
firmware latency; a remote_dma replacement needs routing ids
unavailable client-side), top levels ~10 us.
"""

import os
import sys

import numpy as np

for _p in ("/opt/trn_rl_repo",):
    if _p not in sys.path and os.path.isdir(_p):
        sys.path.insert(0, _p)

import concourse.bacc as bacc
import concourse.mybir as mybir
import concourse.tile as tile
from concourse.bass_utils import run_bass_kernel_spmd

P = 128
H = 512
D = 512
DEPTH = 15
NCORES = 8
MT = 512
F32 = mybir.dt.float32
F32R = mybir.dt.float32r
F16 = mybir.dt.float16
F8 = mybir.dt.float8e4
DRMODE = mybir.MatmulPerfMode.DoubleRow
MULT = mybir.AluOpType.mult

WS, XS, HS = 64.0, 16.0, 16.0        # fp8 operand scales: W, x, h
DQ = 1.0 / (WS * XS)                 # psum dequant (x and h share 16)

BIG_LVLS = (14, 13, 12, 11)
SMALL_LVLS = list(range(10, 2, -1))  # 10..3 sharded
M_BIG = {lvl: 2 ** lvl // NCORES for lvl in BIG_LVLS}
M_SM = {lvl: 2 ** lvl // NCORES for lvl in SMALL_LVLS}
XB_OFF = {}
_acc = 0
for _l in BIG_LVLS:
    XB_OFF[_l] = _acc
    _acc += M_BIG[_l]
NBIG = _acc                          # 3840
XS_OFF = {}
_acc = 0
for _l in SMALL_LVLS:
    XS_OFF[_l] = _acc
    _acc += M_SM[_l]
XS_TOP = _acc                        # 255; then 7 top nodes
NSM = _acc + 7                       # 262

SIG = mybir.ActivationFunctionType.Sigmoid
TANH = mybir.ActivationFunctionType.Tanh
IDENT = mybir.ActivationFunctionType.Identity

_CACHE = {}


def _build_nc(repeat=1, bench_dummy=False, sim1=False, stop_after=None, serialize=False):
    nc = bacc.Bacc("TRN2", target_bir_lowering=False, debug=False,
                   num_devices=1 if sim1 else NCORES)

    kw = {} if bench_dummy else {"kind": "ExternalInput"}
    xt8 = nc.dram_tensor("xt8", [P, 2, 2, NBIG], F8, **kw)
    xt16 = nc.dram_tensor("xt16", [P, 4, NSM], F16, **kw)
    w8i = nc.dram_tensor("w8i", [P, 4, 2, H], F8, **kw)
    w8o = nc.dram_tensor("w8o", [P, 4, 2, H], F8, **kw)
    w8u = nc.dram_tensor("w8u", [P, 4, 2, H], F8, **kw)
    w8fx = nc.dram_tensor("w8fx", [P, 2, 2, H], F8, **kw)
    w8fh = nc.dram_tensor("w8fh", [P, 2, 2, H], F8, **kw)
    # fp16 x-part weights for the gx precompute (small levels)
    w16i = nc.dram_tensor("w16i", [P, 4, H], F16, **kw)
    w16o = nc.dram_tensor("w16o", [P, 4, H], F16, **kw)
    w16u = nc.dram_tensor("w16u", [P, 4, H], F16, **kw)
    w16fx = nc.dram_tensor("w16fx", [P, 4, H], F16, **kw)
    # fp16 h-part weights for the small levels
    wih = nc.dram_tensor("wih", [P, 4, H], F16, **kw)
    woh = nc.dram_tensor("woh", [P, 4, H], F16, **kw)
    wuh = nc.dram_tensor("wuh", [P, 4, H], F16, **kw)
    wfh16 = nc.dram_tensor("wfh16", [P, 4, H], F16, **kw)
    bias = nc.dram_tensor("bias", [P, 16], F32, **kw)
    hc_out = nc.dram_tensor("hc_out", [2, 4, P], F32, kind="ExternalOutput")

    with tile.TileContext(nc) as tc:
        with (
            tc.tile_pool(name="wpool", bufs=1) as wpool,
            tc.tile_pool(name="hbuf", bufs=1) as hbuf,
            tc.tile_pool(name="hsum", bufs=1) as hsump,
            tc.tile_pool(name="h16", bufs=1) as h16p,
            tc.tile_pool(name="g16", bufs=2) as g16p,
            tc.tile_pool(name="gt", bufs=2) as gt,
            tc.tile_pool(name="tmp", bufs=2) as tmp,
            tc.tile_pool(name="fi", bufs=1) as fip,
            tc.tile_pool(name="ps2", bufs=2, space="PSUM") as ps2,
            tc.tile_pool(name="ps1", bufs=1, space="PSUM") as ps1,
            tc.tile_pool(name="dram", bufs=1, space="DRAM") as dram,
        ):
            bias_s = wpool.tile([P, 16], F32, tag="bias")
            nc.sync.dma_start(bias_s[:], bias[:])

            # SBUF-resident inputs.  Cold-start issue order matters if
            # DMAs share a queue: the leaf's first matmuls need w8i + the
            # first x8 chunk, so interleave those ahead of the rest.
            x8_s = wpool.tile([P, 2, 2, NBIG], F8, tag="x8", name="x8_s")
            x16_s = wpool.tile([P, 4, NSM], F16, tag="x16", name="x16_s")

            def load_x(first):
                qs = ((0,) if first else (1, 2, 3))
                for q in qs:
                    c0 = q * (NBIG // 4)
                    nc.sync.dma_start(x8_s[:, :, :, c0:c0 + NBIG // 4],
                                      xt8[:, :, :, c0:c0 + NBIG // 4])
                if not first:
                    nc.sync.dma_start(x16_s[:], xt16[:])

            _x0_loaded = [False]

            def load_w8():
                ws = {}
                for gi, (nm, t, kdn) in enumerate(
                        (("i", w8i, 4), ("o", w8o, 4), ("u", w8u, 4),
                         ("fx", w8fx, 2), ("fh", w8fh, 2))):
                    s = wpool.tile([P, kdn, 2, H], F8, tag=f"w8{nm}",
                                   name=f"w8{nm}_s")
                    for kd in range(kdn):
                        nc.sync.dma_start(s[:, kd], t[:, kd])
                    ws[nm] = s
                    if gi == 0 and not _x0_loaded[0]:
                        load_x(first=True)      # x8 chunk 0 right after w8i
                        _x0_loaded[0] = True
                return ws

            def load_w16():
                ws = {}
                for nm, t in (("i", w16i), ("o", w16o), ("u", w16u),
                              ("fx", w16fx), ("ih", wih), ("oh", woh),
                              ("uh", wuh), ("fh", wfh16)):
                    s = wpool.tile([P, 4, H], F16, tag=f"w16{nm}",
                                   name=f"w16{nm}_s")
                    for ko in range(4):
                        nc.sync.dma_start(s[:, ko], t[:, ko])
                    ws[nm] = s
                return ws

            # level buffers
            h8A = hbuf.tile([P, 4, 1024], F8, tag="h8A")
            h8B = hbuf.tile([P, 4, 1024], F8, tag="h8B")
            cA = hbuf.tile([P, 4, 1024], F32, tag="cA")
            cB = hbuf.tile([P, 4, 1024], F32, tag="cB")
            hA = hbuf.tile([P, 4, 256], F16, tag="hA")
            hB = hbuf.tile([P, 4, 256], F16, tag="hB")
            cA2 = hbuf.tile([P, 4, 256], F32, tag="cA2")
            cB2 = hbuf.tile([P, 4, 256], F32, tag="cB2")
            h3g = hbuf.tile([P, 4, 8], F32R, tag="h3g")
            c3g = hbuf.tile([P, 4, 8], F32, tag="c3g")
            gx = {}
            for g in ("i", "o", "u", "f"):
                gx[g] = hbuf.tile([P, 4, NSM], F16, tag=f"gx{g}",
                                  name=f"gx_{g}")

            def g16_tile(tag, m):
                t = g16p.tile([P, MT], F16, tag=tag, name="g16_" + tag)
                return t[:, :m]

            def tmp_tile(m, nm):
                t = tmp.tile([P, MT], F32, tag="t", name=nm)
                return t[:, :m]

            def tmp_tile3(m, nm):
                t = tmp.tile([P, MT], F32, tag="t", name=nm)
                return t.rearrange("p (ho m) -> p ho m", ho=4)[:, :, :m]

            def leaf_half(w8, xc0, out_c, out_h8):
                """1024 leaf cols in 2 subtiles sharing stationaries."""
                for mo in range(4):
                    ms = slice(mo * P, (mo + 1) * P)
                    o2 = g16p.tile([P, 2, MT], F16, tag="fp", name="o2")
                    for g, bcol in (("i", 0), ("o", 4), ("u", 8)):
                        w = w8[g]
                        pj = [ps2.tile([P, MT], F32, tag=g,
                                       name=f"ps_{g}{j}") for j in (0, 1)]
                        for kd in range(2):
                            for j in (0, 1):
                                c0 = xc0 + j * MT
                                nc.tensor.matmul(
                                    pj[j][:], w[:, kd, :, ms],
                                    x8_s[:, kd, :, c0:c0 + MT],
                                    start=(kd == 0), stop=(kd == 1),
                                    perf_mode=DRMODE)
                        for j in (0, 1):
                            if g == "o":
                                g16 = o2[:, j]
                            else:
                                g16 = g16_tile(g, MT) if j == 0 else \
                                    g16_tile(g + "b", MT)
                            fn = TANH if g == "u" else SIG
                            nc.scalar.activation(
                                g16[:], pj[j][:], fn, scale=DQ,
                                bias=bias_s[:, bcol + mo:bcol + mo + 1])
                            w8[f"_{g}{j}"] = g16
                    oc0 = xc0 % 1024
                    for j in (0, 1):
                        i16 = w8[f"_i{j}"]
                        u16 = w8[f"_u{j}"]
                        c_sl = out_c[:, mo, oc0 + j * MT:oc0 + (j + 1) * MT]
                        nc.vector.tensor_mul(c_sl, i16[:], u16[:])
                    tt2 = g16p.tile([P, 2, MT], F16, tag="tt2", name="tt2")
                    nc.scalar.activation(
                        tt2.rearrange("p a b -> p (a b)"),
                        out_c[:, mo, oc0:oc0 + 2 * MT], TANH)
                    nc.vector.scalar_tensor_tensor(
                        out_h8[:, mo, oc0:oc0 + 2 * MT],
                        tt2.rearrange("p a b -> p (a b)"), HS,
                        o2.rearrange("p a b -> p (a b)"), MULT, MULT)

            def big_internal(w8, xc0, m, ch_h8, ch_c, out_c, oc0,
                             out_h8=None, out_h=None):
                """Internal fp8 level tile (m<=512 cols).  Children at
                ch_h8/ch_c cols 0..2m.  h out: fp8 (out_h8) or f32r."""
                hs8 = hsump.tile([P, 4, MT], F8, tag="hs8",
                                 name="hs8")[:, :, :m]
                nc.vector.tensor_add(hs8[:], ch_h8[:, :, 0:2 * m:2],
                                     ch_h8[:, :, 1:2 * m:2])
                for mo in range(4):
                    ms = slice(mo * P, (mo + 1) * P)
                    ps = {}
                    for g in ("i", "o", "u"):
                        p = ps2.tile([P, MT], F32, tag=g,
                                     name=f"ps_{g}")[:, :m]
                        w = w8[g]
                        for kd in range(2):
                            nc.tensor.matmul(
                                p[:], w[:, kd, :, ms],
                                x8_s[:, kd, :, xc0:xc0 + m],
                                start=(kd == 0), stop=False,
                                perf_mode=DRMODE)
                        for kd in range(2):
                            nc.tensor.matmul(
                                p[:], w[:, 2 + kd, :, ms],
                                hs8[:, 2 * kd:2 * kd + 2],
                                start=False, stop=(kd == 1),
                                perf_mode=DRMODE)
                        ps[g] = p
                    pf = ps1.tile([P, 2, MT], F32, tag="fl",
                                  name="ps_f2")[:, :, :m]
                    pfl = pf[:, 0]
                    pfr = pf[:, 1]
                    for kd in range(2):
                        w = w8["fx"][:, kd, :, ms]
                        xsl = x8_s[:, kd, :, xc0:xc0 + m]
                        nc.tensor.matmul(pfl[:], w, xsl, start=(kd == 0),
                                         stop=False, perf_mode=DRMODE)
                        nc.tensor.matmul(pfr[:], w, xsl, start=(kd == 0),
                                         stop=False, perf_mode=DRMODE)
                    for kd in range(2):
                        w = w8["fh"][:, kd, :, ms]
                        nc.tensor.matmul(
                            pfl[:], w, ch_h8[:, 2 * kd:2 * kd + 2, 0:2 * m:2],
                            start=False, stop=(kd == 1), perf_mode=DRMODE)
                        nc.tensor.matmul(
                            pfr[:], w, ch_h8[:, 2 * kd:2 * kd + 2, 1:2 * m:2],
                            start=False, stop=(kd == 1), perf_mode=DRMODE)
                    i16 = g16_tile("i", m)
                    o16 = g16_tile("o", m)
                    u16 = g16_tile("u", m)
                    f16_pair = g16p.tile([P, 2, MT], F16, tag="fp",
                                         name="f16_pair")[:, :, :m]
                    fl16 = f16_pair[:, 0]
                    fr16 = f16_pair[:, 1]
                    nc.scalar.activation(i16[:], ps["i"][:], SIG, scale=DQ,
                                         bias=bias_s[:, mo:mo + 1])
                    nc.scalar.activation(o16[:], ps["o"][:], SIG, scale=DQ,
                                         bias=bias_s[:, 4 + mo:5 + mo])
                    nc.scalar.activation(u16[:], ps["u"][:], TANH, scale=DQ,
                                         bias=bias_s[:, 8 + mo:9 + mo])
                    nc.scalar.activation(f16_pair[:], pf[:], SIG, scale=DQ,
                                         bias=bias_s[:, 12 + mo:13 + mo])
                    c_sl = out_c[:, mo, oc0:oc0 + m]
                    nc.vector.tensor_mul(c_sl, i16[:], u16[:])
                    t1 = tmp_tile(m, "t1")
                    nc.vector.tensor_mul(t1[:], fl16,
                                         ch_c[:, mo, 0:2 * m:2])
                    nc.vector.tensor_add(c_sl, c_sl, t1[:])
                    t2 = tmp_tile(m, "t2")
                    nc.vector.tensor_mul(t2[:], fr16,
                                         ch_c[:, mo, 1:2 * m:2])
                    nc.vector.tensor_add(c_sl, c_sl, t2[:])
                    tt = g16_tile("tt", m)
                    nc.scalar.activation(tt[:], c_sl, TANH)
                    if out_h8 is not None:
                        nc.vector.scalar_tensor_tensor(
                            out_h8[:, mo, oc0:oc0 + m], tt[:], HS, o16[:],
                            MULT, MULT)
                    else:
                        nc.vector.tensor_mul(out_h[:, mo, 0:m], o16[:],
                                             tt[:])

            def precompute_gx(w16):
                """Batched fp16 x-parts (+bias) for small+top nodes."""
                for mo in range(4):
                    ms = slice(mo * P, (mo + 1) * P)
                    for gi, (g, wnm, bcol) in enumerate(
                            (("i", "i", 0), ("o", "o", 4),
                             ("u", "u", 8), ("f", "fx", 12))):
                        w_s = w16[wnm]
                        ps = ps2.tile([P, MT], F32,
                                      tag=("i", "o", "u")[gi % 3],
                                      name="ps_pre")[:, :NSM]
                        for ko in range(4):
                            nc.tensor.matmul(
                                ps[:], w_s[:, ko, ms], x16_s[:, ko],
                                start=(ko == 0), stop=(ko == 3))
                        nc.scalar.activation(
                            gx[g][:, mo], ps[:], IDENT,
                            bias=bias_s[:, bcol + mo:bcol + mo + 1])

            def small_level(w16, m, gxoff, ch_h, ch_c, out_h, out_c):
                """Levels m<=128: fp16 h-part matmuls; x-parts from gx.
                m=1 levels padded to 2."""
                m = max(m, 2)
                hs = h16p.tile([P, 4, P], F16, tag="hs16",
                               name="hs")[:, :, :m]
                nc.vector.tensor_add(hs[:], ch_h[:, :, 0:2 * m:2],
                                     ch_h[:, :, 1:2 * m:2])
                if ch_h.dtype == F16:
                    hch = ch_h[:, :, 0:2 * m]
                else:
                    hch_t = h16p.tile([P, 4, 2 * P], F16, tag="hch16",
                                      name="hch")[:, :, :2 * m]
                    nc.vector.tensor_copy(hch_t[:], ch_h[:, :, 0:2 * m])
                    hch = hch_t[:]
                ps_i = ps2.tile([P, 4, P], F32, tag="i",
                                name="ps_i")[:, :, :m]
                ps_u = ps2.tile([P, 4, P], F32, tag="u",
                                name="ps_u")[:, :, :m]
                ps_o = ps2.tile([P, 4, P], F32, tag="o",
                                name="ps_o")[:, :, :m]
                gsl = slice(gxoff, gxoff + m)
                f_sb = fip.tile([P, 4, 2 * P], F32, tag="fint",
                                name="f_sb")[:, :, :2 * m]
                for mo in range(4):
                    ms = slice(mo * P, (mo + 1) * P)
                    psf = ps1.tile([P, 2, MT], F32, tag="fl",
                                   name="ps_f")[:, mo % 2, :2 * m]
                    for ko in range(4):
                        nc.tensor.matmul(
                            psf[:], w16["fh"][:, ko, ms], hch[:, ko],
                            start=(ko == 0), stop=(ko == 3))
                    nc.vector.tensor_add(
                        f_sb[:, mo].rearrange("p (m two) -> p m two", two=2),
                        psf.rearrange("p (m two) -> p m two", two=2),
                        gx["f"][:, mo, gsl, None].to_broadcast((P, m, 2)))
                for mo in range(4):
                    ms = slice(mo * P, (mo + 1) * P)
                    for wnm, ps in (("ih", ps_i), ("oh", ps_o),
                                    ("uh", ps_u)):
                        w_s = w16[wnm]
                        for ko in range(4):
                            nc.tensor.matmul(
                                ps[:, mo], w_s[:, ko, ms], hs[:, ko],
                                start=(ko == 0), stop=(ko == 3))
                nc.scalar.activation(f_sb[:], f_sb[:], SIG)
                io = gt.tile([P, 2, MT], F32, tag="io",
                             name="g_io").rearrange(
                    "p t (ho m) -> p t ho m", ho=4)[:, :, :, :m]
                nc.vector.tensor_add(io[:, 0], ps_i[:], gx["i"][:, :, gsl])
                nc.vector.tensor_add(io[:, 1], ps_o[:], gx["o"][:, :, gsl])
                nc.scalar.activation(io[:], io[:], SIG)
                u_sb = gt.tile([P, MT], F32, tag="u",
                               name="g_u").rearrange(
                    "p (ho m) -> p ho m", ho=4)[:, :, :m]
                nc.vector.tensor_add(u_sb[:], ps_u[:], gx["u"][:, :, gsl])
                nc.scalar.activation(u_sb[:], u_sb[:], TANH)
                gates = {"i": io[:, 0], "o": io[:, 1], "u": u_sb}
                c_sl = out_c[:, :, 0:m]
                h_sl = out_h[:, :, 0:m]
                c_l = ch_c[:, :, 0:2 * m:2]
                c_r = ch_c[:, :, 1:2 * m:2]
                nc.vector.tensor_mul(c_sl, gates["i"], gates["u"][:])
                t1 = tmp_tile3(m, "t1")
                nc.vector.tensor_mul(t1[:], f_sb[:, :, 0::2], c_l)
                nc.vector.tensor_add(c_sl, c_sl, t1[:])
                t2 = tmp_tile3(m, "t2")
                nc.vector.tensor_mul(t2[:], f_sb[:, :, 1::2], c_r)
                nc.vector.tensor_add(c_sl, c_sl, t2[:])
                tt = tmp_tile3(m, "tt")
                nc.scalar.activation(tt[:], c_sl, TANH)
                nc.vector.tensor_mul(h_sl, gates["o"], tt[:])

            if repeat == 0:
                nc.sync.dma_start(
                    hc_out[:],
                    xt8.bitcast(F32)[0:2].rearrange(
                        "a kd i n -> a (kd i) n")[:, :, 0:P])
            _x_rest_loaded = [False]
            for _rep in range(repeat):
                if serialize and _rep > 0:
                    # 1-elem token per 512-col subtile: rep N's first MMs
                    # depend on rep N-1's final hA (true serial latency)
                    nc.vector.scalar_tensor_tensor(
                        x8_s[:, 0, 0, 0:NBIG:512], hA[:, 0, 0:8], 0.0,
                        x8_s[:, 0, 0, 0:NBIG:512], MULT,
                        mybir.AluOpType.add)
                w8 = load_w8()
                if not _x_rest_loaded[0]:
                    load_x(first=False)
                    _x_rest_loaded[0] = True
                w16 = load_w16()
                with nc.named_scope("L14h0"):
                    leaf_half(w8, 0, cA, h8A)
                with nc.named_scope("pre"):
                    precompute_gx(w16)
                with nc.named_scope("L13j0"):
                    big_internal(w8, XB_OFF[13], 512, h8A, cA, cB, 0,
                                 out_h8=h8B)
                with nc.named_scope("L14h1"):
                    leaf_half(w8, 1024, cA, h8A)
                with nc.named_scope("L13j1"):
                    big_internal(w8, XB_OFF[13] + 512, 512, h8A, cA, cB, 512,
                                 out_h8=h8B)
                with nc.named_scope("L12"):
                    big_internal(w8, XB_OFF[12], 512, h8B, cB, cA, 0,
                                 out_h8=h8A)
                with nc.named_scope("L11"):
                    big_internal(w8, XB_OFF[11], 256, h8A, cA, cA2, 0,
                                 out_h=hA)
                if stop_after == "L11":
                    hf11 = h16p.tile([P, 4, 1], F32, tag="hfin",
                                     name="hf11")
                    nc.vector.tensor_copy(hf11[:], hA[:, :, 0:1])
                    nc.sync.dma_start(
                        hc_out[0:1].rearrange("one ko p -> p ko one"),
                        hf11[:])
                    nc.sync.dma_start(
                        hc_out[1:2].rearrange("one ko p -> p ko one"),
                        cA2[:, :, 0:1])
                    continue
                # small levels 10..3: ping-pong (hA,cA2) <-> (hB,cB2)
                cur_h, cur_c = hA, cA2
                for lvl in SMALL_LVLS:
                    nxt_h = hB if cur_h is hA else hA
                    nxt_c = cB2 if cur_c is cA2 else cA2
                    with nc.named_scope(f"L{lvl}"):
                        small_level(w16, M_SM[lvl], XS_OFF[lvl],
                                    cur_h, cur_c, nxt_h, nxt_c)
                    cur_h, cur_c = nxt_h, nxt_c

                if stop_after == "fakegather":
                    with nc.named_scope("fakegather"):
                        for r in range(NCORES):
                            nc.sync.dma_start(h3g[:, :, r:r + 1],
                                              cur_h[:, :, 0:1])
                            nc.sync.dma_start(c3g[:, :, r:r + 1],
                                              cur_c[:, :, 0:1])
                    with nc.named_scope("L2f"):
                        small_level(w16, 4, XS_TOP + 3, h3g, c3g, hA, cA2)
                    with nc.named_scope("L1f"):
                        small_level(w16, 2, XS_TOP + 1, hA, cA2, hB, cB2)
                    with nc.named_scope("L0f"):
                        small_level(w16, 1, XS_TOP + 0, hB, cB2, hA, cA2)
                    hff = h16p.tile([P, 4, 1], F32, tag="hfin", name="hff")
                    nc.vector.tensor_copy(hff[:], hA[:, :, 0:1])
                    nc.sync.dma_start(
                        hc_out[0:1].rearrange("one ko p -> p ko one"),
                        hff[:])
                    nc.sync.dma_start(
                        hc_out[1:2].rearrange("one ko p -> p ko one"),
                        cA2[:, :, 0:1])
                    continue
                if stop_after == "L3":
                    hf3 = h16p.tile([P, 4, 1], F32, tag="hfin", name="hf3")
                    nc.vector.tensor_copy(hf3[:], cur_h[:, :, 0:1])
                    nc.sync.dma_start(
                        hc_out[0:1].rearrange("one ko p -> p ko one"),
                        hf3[:])
                    nc.sync.dma_start(
                        hc_out[1:2].rearrange("one ko p -> p ko one"),
                        cur_c[:, :, 0:1])
                    continue
                with nc.named_scope("gather"):
                    cc_in = dram.tile([2, 4, P], F32R, name="cc_in")
                    cc_out = dram.tile([8, 2, 4, P], F32R, name="cc_out")
                    h3f = h16p.tile([P, 4, 1], F32R, tag="h3f",
                                    name="h3f")
                    nc.vector.tensor_copy(h3f[:], cur_h[:, :, 0:1])
                    nc.sync.dma_start(
                        cc_in[0:1].rearrange("one ko p -> p ko one"),
                        h3f[:])
                    nc.sync.dma_start(
                        cc_in[1:2].rearrange("one ko p -> p ko one"),
                        cur_c.bitcast(F32R)[:, :, 0:1])
                    if sim1:
                        for r in range(NCORES):
                            nc.sync.dma_start(cc_out[r], cc_in[:])
                    else:
                        nc.gpsimd.collective_compute(
                            "AllGather", mybir.AluOpType.bypass,
                            replica_groups=[list(range(NCORES))],
                            ins=[cc_in.opt()], outs=[cc_out.opt()])
                    for ko in range(4):
                        nc.sync.dma_start(
                            h3g[:, ko, 0:8],
                            cc_out[:, 0, ko].rearrange("r p -> p r"))
                        nc.sync.dma_start(
                            c3g[:, ko, 0:8],
                            cc_out.bitcast(F32)[:, 1, ko].rearrange(
                                "r p -> p r"))

                with nc.named_scope("L2"):
                    small_level(w16, 4, XS_TOP + 3, h3g, c3g, hA, cA2)
                with nc.named_scope("L1"):
                    small_level(w16, 2, XS_TOP + 1, hA, cA2, hB, cB2)
                with nc.named_scope("L0"):
                    small_level(w16, 1, XS_TOP + 0, hB, cB2, hA, cA2)

                hfin = h16p.tile([P, 4, 1], F32, tag="hfin", name="hfin")
                nc.vector.tensor_copy(hfin[:], hA[:, :, 0:1])
                nc.sync.dma_start(
                    hc_out[0:1].rearrange("one ko p -> p ko one"),
                    hfin[:])
                nc.sync.dma_start(
                    hc_out[1:2].rearrange("one ko p -> p ko one"),
                    cA2[:, :, 0:1])

    nc.compile()
    return nc


def _prep_inputs(x, Wi, bi, Wf, bf, Wo, bo, Wu, bu):
    import ml_dtypes
    E4 = ml_dtypes.float8_e4m3
    Wi, Wf, Wo, Wu = (np.asarray(w, np.float32) for w in (Wi, Wf, Wo, Wu))

    def wt8(wpart):  # [H(M), 512(K)] -> [P, 2(kd), 2, H] fp8 scaled
        a = wpart.T.reshape(2, 2, P, H).transpose(2, 0, 1, 3)
        return np.ascontiguousarray(a * WS).astype(E4)

    def wt16(wpart):  # [H, 512] -> [P, 4(ko), H] fp16
        a = wpart.T.reshape(4, P, H).transpose(1, 0, 2)
        return np.ascontiguousarray(a).astype(np.float16)

    w8 = {}
    for nm, w in (("i", Wi), ("o", Wo), ("u", Wu)):
        w8[nm] = np.concatenate([wt8(w[:, :D]), wt8(w[:, D:])], axis=1)
    w8fx, w8fh = wt8(Wf[:, :D]), wt8(Wf[:, D:])
    w16 = {nm: wt16(w[:, :D]) for nm, w in
           (("i", Wi), ("o", Wo), ("u", Wu), ("fx", Wf))}
    w16h = {nm: wt16(w[:, D:]) for nm, w in
            (("ih", Wi), ("oh", Wo), ("uh", Wu), ("fh", Wf))}
    bias = np.stack(
        [np.asarray(b, np.float32).reshape(4, P) for b in (bi, bo, bu, bf)],
        axis=0)
    bias = np.ascontiguousarray(bias.reshape(16, P).T).astype(np.float32)

    x = np.asarray(x, dtype=np.float32)
    in_maps = []
    for j in range(NCORES):
        bcols = []
        for lvl in BIG_LVLS:
            s, m = 2 ** lvl - 1, M_BIG[lvl]
            bcols.extend(range(s + j * m, s + (j + 1) * m))
        scols = []
        for lvl in SMALL_LVLS:
            s, m = 2 ** lvl - 1, M_SM[lvl]
            scols.extend(range(s + j * m, s + (j + 1) * m))
        scols.extend(range(7))
        xb = x[bcols]                            # [NBIG, 512]
        x8 = np.ascontiguousarray(
            xb.T.reshape(2, 2, P, NBIG).transpose(2, 0, 1, 3) * XS
        ).astype(E4)
        xsm = x[scols]                           # [NSM, 512]
        x16 = np.ascontiguousarray(
            xsm.T.reshape(4, P, NSM).transpose(1, 0, 2)).astype(np.float16)
        in_maps.append({
            "xt8": x8, "xt16": x16,
            "w8i": w8["i"], "w8o": w8["o"], "w8u": w8["u"],
            "w8fx": w8fx, "w8fh": w8fh,
            "w16i": w16["i"], "w16o": w16["o"], "w16u": w16["u"],
            "w16fx": w16["fx"],
            "wih": w16h["ih"], "woh": w16h["oh"], "wuh": w16h["uh"],
            "wfh16": w16h["fh"], "bias": bias,
        })
    return in_maps


def _make_runner(nc, n_cores=NCORES):
    """Build the sharded jitted callable once (mirrors
    bass2jax.run_bass_via_pjrt) so repeated timed calls don't recompile."""
    import jax
    from jax.sharding import Mesh, PartitionSpec
    from jax.experimental.shard_map import shard_map
    from concourse import bass2jax
    from concourse.bass2jax import _bass_exec_p, install_neuronx_cc_hook

    install_neuronx_cc_hook()
    partition_name = (nc.partition_id_tensor.name
                      if nc.partition_id_tensor else None)
    in_names, out_names, out_avals, zero_outs = [], [], [], []
    for alloc in nc.m.functions[0].allocations:
        if not isinstance(alloc, mybir.MemoryLocationSet):
            continue
        name = alloc.memorylocations[0].name
        if alloc.kind == "ExternalInput":
            if name != partition_name:
                in_names.append(name)
        elif alloc.kind == "ExternalOutput":
            shape = tuple(alloc.tensor_shape)
            dtype = mybir.dt.np(alloc.dtype)
            out_names.append(name)
            out_avals.append(jax.core.ShapedArray(shape, dtype))
            zero_outs.append(np.zeros(shape, dtype))
    n_params = len(in_names)
    n_outs = len(out_avals)
    full_in_names = list(in_names) + list(out_names)
    if partition_name is not None:
        full_in_names.append(partition_name)

    def _body(*args):
        operands = list(args)
        if partition_name is not None:
            operands.append(bass2jax.partition_id_tensor())
        outs = _bass_exec_p.bind(
            *operands,
            out_avals=tuple(out_avals),
            in_names=tuple(full_in_names),
            out_names=tuple(out_names),
            lowering_input_output_aliases=(),
            sim_require_finite=True,
            sim_require_nnan=True,
            nc=nc,
        )
        return tuple(outs)

    devices = jax.devices()[:n_cores]
    mesh = Mesh(np.asarray(devices), ("core",))
    in_specs = (PartitionSpec("core"),) * (n_params + n_outs)
    out_specs = (PartitionSpec("core"),) * n_outs
    donate = tuple(range(n_params, n_params + n_outs))
    sharded = jax.jit(
        shard_map(_body, mesh=mesh, in_specs=in_specs,
                  out_specs=out_specs, check_rep=False),
        donate_argnums=donate, keep_unused=True)

    def run(in_maps):
        per_core = [[np.asarray(m[name]) for name in in_names]
                    for m in in_maps]
        concat_in = [np.concatenate([per_core[c][i] for c in range(n_cores)],
                                    axis=0) for i in range(n_params)]
        concat_zeros = [np.zeros((n_cores * z.shape[0], *z.shape[1:]),
                                 z.dtype) for z in zero_outs]
        outs = sharded(*concat_in, *concat_zeros)
        jax.block_until_ready(outs)
        return outs

    return run, out_avals


def _make_caller(nc):
    """Non-blocking sharded caller for bench (dummy-input builds)."""
    import jax
    from jax.sharding import Mesh, PartitionSpec
    from jax.experimental.shard_map import shard_map
    from concourse import bass2jax
    from concourse.bass2jax import _bass_exec_p, install_neuronx_cc_hook

    install_neuronx_cc_hook()
    partition_name = (nc.partition_id_tensor.name
                      if nc.partition_id_tensor else None)
    out_names, out_avals, zero_outs = [], [], []
    for alloc in nc.m.functions[0].allocations:
        if not isinstance(alloc, mybir.MemoryLocationSet):
            continue
        if alloc.kind == "ExternalOutput":
            shape = tuple(alloc.tensor_shape)
            dtype = mybir.dt.np(alloc.dtype)
            out_names.append(alloc.memorylocations[0].name)
            out_avals.append(jax.core.ShapedArray(shape, dtype))
            zero_outs.append(np.zeros(shape, dtype))
    full_in_names = list(out_names)
    if partition_name is not None:
        full_in_names.append(partition_name)

    def _body(*args):
        operands = list(args)
        if partition_name is not None:
            operands.append(bass2jax.partition_id_tensor())
        return tuple(_bass_exec_p.bind(
            *operands, out_avals=tuple(out_avals),
            in_names=tuple(full_in_names), out_names=tuple(out_names),
            lowering_input_output_aliases=(), sim_require_finite=True,
            sim_require_nnan=True, nc=nc))

    devices = jax.devices()[:NCORES]
    mesh = Mesh(np.asarray(devices), ("core",))
    n_outs = len(out_avals)
    sharded = jax.jit(
        shard_map(_body, mesh=mesh,
                  in_specs=(PartitionSpec("core"),) * n_outs,
                  out_specs=(PartitionSpec("core"),) * n_outs,
                  check_rep=False),
        donate_argnums=tuple(range(n_outs)), keep_unused=True)

    def call():
        czeros = [np.zeros((NCORES * z.shape[0], *z.shape[1:]), z.dtype)
                  for z in zero_outs]
        return sharded(*czeros)
    return call


def bench(reps=(2, 18), iters=40, stop_after=None, serialize=True,
          batches=8):
    """Async-pipelined, batch-interleaved delta timing."""
    import time
    import jax
    calls = []
    for rep in reps:
        nc = _build_nc(repeat=rep, bench_dummy=True,
                       stop_after=stop_after, serialize=serialize)
        call = _make_caller(nc)
        jax.block_until_ready(call())
        calls.append(call)

    def batch(call):
        t0 = time.perf_counter()
        outs = [call() for _ in range(iters)]
        jax.block_until_ready(outs)
        return (time.perf_counter() - t0) / iters

    batch(calls[0]); batch(calls[1])  # extra warm
    diffs = []
    for k in range(batches):
        a, b = (0, 1) if k % 2 == 0 else (1, 0)
        ta = batch(calls[a])
        tb = batch(calls[b])
        d = (tb - ta) if a == 0 else (ta - tb)
        diffs.append(d)
    diffs.sort()
    n = len(diffs)
    mid = diffs[n // 4: n - n // 4] or diffs
    med = sum(mid) / len(mid)
    print(f"  bench diffs(ms): {[f'{d*1e3:.2f}' for d in diffs]}")
    return med / (reps[1] - reps[0]) * 1e9


def kernel(x, Wi, bi, Wf, bf, Wo, bo, Wu, bu):
    if "nc" not in _CACHE:
        _CACHE["nc"] = _build_nc()
    nc = _CACHE["nc"]
    in_maps = _prep_inputs(x, Wi, bi, Wf, bf, Wo, bo, Wu, bu)
    res = run_bass_kernel_spmd(nc, in_maps, core_ids=list(range(NCORES)))
    out = res.results[0]["hc_out"]               # [2, 4, 128]
    h0 = np.ascontiguousarray(out[0].reshape(H)).astype(np.float32)
    c0 = np.ascontiguousarray(out[1].reshape(H)).astype(np.float32)
    return h0, c0


# revision 16
# speedup vs baseline: 1.1172x; 1.1172x over previous
"""Child-Sum Tree-LSTM (complete binary tree, depth 15, heap layout) on
8 Trainium2 cores.

Sharding (per hint): data-parallel over nodes within each level.  The
heap layout makes per-level chunks self-contained: core j's chunk at
level l has its children exactly in core j's chunk at level l+1, so
levels 14..3 run with zero communication.  One AllGather ships the 8
level-3 (h,c) pairs to every core; levels 2..0 (7 nodes) are computed
replicated.

Compute recipe (dtype mix validated by numpy emulation, ~6e-3 rel err
vs the 2e-2 gate):
  - Levels 14..11 ("big", >=256 nodes/core): all matmuls fp8e4m3 with
    DoubleRow perf mode (K=256 per MM, ~2x effective rate over
    bf16/fp32r: measured 164-189 ns vs 202 ns per 512-wide MM at half
    the MM count).  Operand scaling x*16, h*16, W*64; psum dequant
    1/1024 via ACT scale.  h of levels 14..12 is stored fp8*16 only
    (consumed only by the next level's matmuls); c stays fp32; gates
    come out of ACT in fp16 so the DVE elementwise runs in 2x SBUF
    mode.  Leaf halves interleave with level-13 halves (ping-pong c/h8
    buffers); leaf stationaries are reused across two 512-col subtiles.
  - Levels 10..0 ("small", <=128 nodes/core): latency-bound, so their
    x-parts (+bias) for all 262 small+top nodes are precomputed in one
    batched fp16 pass (gx); each level then runs only the recurrent
    h-part matmuls in fp16 (48 for i/o/u + 16 for f with left/right
    children interleaved in the moving operand).  h buffers are fp16
    (the f-matmuls stream child h directly, no cast copy); forget-gate
    psums ping-pong between two banks so consecutive mo-chunks do not
    serialize.
  - Level 11 bridges the two: fp8 inputs, fp16 h output.

  - Forget-gate psums live in one paired 2-bank tile so fl/fr activate
    in a single ACT op; leaf tanh(c) and the h8 write are batched across
    the two 512-col subtiles (the big phase is ACT-limited, so fewer,
    larger ACT ops matter).  Within a small level the f matmuls issue
    before the i/o/u ones (they read child h directly, no wait on the
    child-sum), and the i/o gates pair into one tile for a single
    sigmoid ACT.

Timing on this host (serialized repeat deltas, async-pipelined calls —
wall-clock per call is ~100 ms of tunnel overhead that completely hides
device time, so only in-NEFF repeat deltas are meaningful): full tree
~171 us/core vs ~270-283 us for the staged fp32r baseline (~1.6x).
Attribution by stop_after ablation: big levels ~70 us (PE/ACT
balanced), small levels ~45 us, AllGather collective ~36 us (fixed
firmware latency; a remote_dma replacement needs routing ids
unavailable client-side), top levels ~10 us.
"""

import os
import sys

import numpy as np

for _p in ("/opt/trn_rl_repo",):
    if _p not in sys.path and os.path.isdir(_p):
        sys.path.insert(0, _p)

import concourse.bacc as bacc
import concourse.mybir as mybir
import concourse.tile as tile
from concourse.bass_utils import run_bass_kernel_spmd

P = 128
H = 512
D = 512
DEPTH = 15
NCORES = 8
MT = 512
F32 = mybir.dt.float32
F32R = mybir.dt.float32r
F16 = mybir.dt.float16
F8 = mybir.dt.float8e4
DRMODE = mybir.MatmulPerfMode.DoubleRow
MULT = mybir.AluOpType.mult

WS, XS, HS = 64.0, 16.0, 16.0        # fp8 operand scales: W, x, h
DQ = 1.0 / (WS * XS)                 # psum dequant (x and h share 16)

BIG_LVLS = (14, 13, 12, 11)
SMALL_LVLS = list(range(10, 2, -1))  # 10..3 sharded
M_BIG = {lvl: 2 ** lvl // NCORES for lvl in BIG_LVLS}
M_SM = {lvl: 2 ** lvl // NCORES for lvl in SMALL_LVLS}
XB_OFF = {}
_acc = 0
for _l in BIG_LVLS:
    XB_OFF[_l] = _acc
    _acc += M_BIG[_l]
NBIG = _acc                          # 3840
XS_OFF = {}
_acc = 0
for _l in SMALL_LVLS:
    XS_OFF[_l] = _acc
    _acc += M_SM[_l]
XS_TOP = _acc                        # 255; then 7 top nodes
NSM = _acc + 7                       # 262

SIG = mybir.ActivationFunctionType.Sigmoid
TANH = mybir.ActivationFunctionType.Tanh
IDENT = mybir.ActivationFunctionType.Identity

_CACHE = {}


def _build_nc(repeat=1, bench_dummy=False, sim1=False, stop_after=None, serialize=False):
    nc = bacc.Bacc("TRN2", target_bir_lowering=False, debug=False,
                   num_devices=1 if sim1 else NCORES)

    kw = {} if bench_dummy else {"kind": "ExternalInput"}
    xt8 = nc.dram_tensor("xt8", [P, 2, 2, NBIG], F8, **kw)
    xt16 = nc.dram_tensor("xt16", [P, 4, NSM], F16, **kw)
    w8i = nc.dram_tensor("w8i", [P, 4, 2, H], F8, **kw)
    w8o = nc.dram_tensor("w8o", [P, 4, 2, H], F8, **kw)
    w8u = nc.dram_tensor("w8u", [P, 4, 2, H], F8, **kw)
    w8fx = nc.dram_tensor("w8fx", [P, 2, 2, H], F8, **kw)
    w8fh = nc.dram_tensor("w8fh", [P, 2, 2, H], F8, **kw)
    # fp16 x-part weights for the gx precompute (small levels)
    w16i = nc.dram_tensor("w16i", [P, 4, H], F16, **kw)
    w16o = nc.dram_tensor("w16o", [P, 4, H], F16, **kw)
    w16u = nc.dram_tensor("w16u", [P, 4, H], F16, **kw)
    w16fx = nc.dram_tensor("w16fx", [P, 4, H], F16, **kw)
    # fp16 h-part weights for the small levels
    wih = nc.dram_tensor("wih", [P, 4, H], F16, **kw)
    woh = nc.dram_tensor("woh", [P, 4, H], F16, **kw)
    wuh = nc.dram_tensor("wuh", [P, 4, H], F16, **kw)
    wfh16 = nc.dram_tensor("wfh16", [P, 4, H], F16, **kw)
    bias = nc.dram_tensor("bias", [P, 16], F32, **kw)
    hc_out = nc.dram_tensor("hc_out", [2, 4, P], F32, kind="ExternalOutput")

    with tile.TileContext(nc) as tc:
        with (
            tc.tile_pool(name="wpool", bufs=1) as wpool,
            tc.tile_pool(name="hbuf", bufs=1) as hbuf,
            tc.tile_pool(name="hsum", bufs=1) as hsump,
            tc.tile_pool(name="h16", bufs=1) as h16p,
            tc.tile_pool(name="g16", bufs=2) as g16p,
            tc.tile_pool(name="gt", bufs=2) as gt,
            tc.tile_pool(name="tmp", bufs=2) as tmp,
            tc.tile_pool(name="fi", bufs=1) as fip,
            tc.tile_pool(name="ps2", bufs=2, space="PSUM") as ps2,
            tc.tile_pool(name="ps1", bufs=1, space="PSUM") as ps1,
            tc.tile_pool(name="dram", bufs=1, space="DRAM") as dram,
        ):
            bias_s = wpool.tile([P, 16], F32, tag="bias")
            nc.sync.dma_start(bias_s[:], bias[:])

            # SBUF-resident inputs.  Cold-start issue order matters if
            # DMAs share a queue: the leaf's first matmuls need w8i + the
            # first x8 chunk, so interleave those ahead of the rest.
            x8_s = wpool.tile([P, 2, 2, NBIG], F8, tag="x8", name="x8_s")
            x16_s = wpool.tile([P, 4, NSM], F16, tag="x16", name="x16_s")

            # chunk boundaries align with consumer phases: leaf half 0,
            # leaf half 1, L13, L12+L11
            _XCHUNKS = ((0, 1024), (1024, 2048), (2048, 3072), (3072, NBIG))

            def load_x(first):
                for c0, c1 in (_XCHUNKS[:1] if first else _XCHUNKS[1:]):
                    nc.sync.dma_start(x8_s[:, :, :, c0:c1],
                                      xt8[:, :, :, c0:c1])
                if not first:
                    nc.sync.dma_start(x16_s[:], xt16[:])

            _x0_loaded = [False]

            def load_w8():
                ws = {}
                for gi, (nm, t, kdn) in enumerate(
                        (("i", w8i, 4), ("o", w8o, 4), ("u", w8u, 4),
                         ("fx", w8fx, 2), ("fh", w8fh, 2))):
                    s = wpool.tile([P, kdn, 2, H], F8, tag=f"w8{nm}",
                                   name=f"w8{nm}_s")
                    for kd in range(kdn):
                        nc.sync.dma_start(s[:, kd], t[:, kd])
                    ws[nm] = s
                    if gi == 0 and not _x0_loaded[0]:
                        load_x(first=True)      # x8 chunk 0 right after w8i
                        _x0_loaded[0] = True
                return ws

            def load_w16():
                ws = {}
                for nm, t in (("i", w16i), ("o", w16o), ("u", w16u),
                              ("fx", w16fx), ("ih", wih), ("oh", woh),
                              ("uh", wuh), ("fh", wfh16)):
                    s = wpool.tile([P, 4, H], F16, tag=f"w16{nm}",
                                   name=f"w16{nm}_s")
                    for ko in range(4):
                        nc.sync.dma_start(s[:, ko], t[:, ko])
                    ws[nm] = s
                return ws

            # level buffers
            h8A = hbuf.tile([P, 4, 1024], F8, tag="h8A")
            h8B = hbuf.tile([P, 4, 1024], F8, tag="h8B")
            cA = hbuf.tile([P, 4, 1024], F32, tag="cA")
            cB = hbuf.tile([P, 4, 1024], F32, tag="cB")
            hA = hbuf.tile([P, 4, 256], F16, tag="hA")
            hB = hbuf.tile([P, 4, 256], F16, tag="hB")
            cA2 = hbuf.tile([P, 4, 256], F32, tag="cA2")
            cB2 = hbuf.tile([P, 4, 256], F32, tag="cB2")
            h3g = hbuf.tile([P, 4, 8], F32R, tag="h3g")
            c3g = hbuf.tile([P, 4, 8], F32, tag="c3g")
            gx = {}
            for g in ("i", "o", "u", "f"):
                gx[g] = hbuf.tile([P, 4, NSM], F16, tag=f"gx{g}",
                                  name=f"gx_{g}")

            def g16_tile(tag, m):
                t = g16p.tile([P, MT], F16, tag=tag, name="g16_" + tag)
                return t[:, :m]

            def tmp_tile(m, nm):
                t = tmp.tile([P, MT], F32, tag="t", name=nm)
                return t[:, :m]

            def tmp_tile3(m, nm):
                t = tmp.tile([P, MT], F32, tag="t", name=nm)
                return t.rearrange("p (ho m) -> p ho m", ho=4)[:, :, :m]

            def leaf_half(w8, xc0, out_c, out_h8):
                """1024 leaf cols in 2 subtiles sharing stationaries."""
                for mo in range(4):
                    ms = slice(mo * P, (mo + 1) * P)
                    o2 = g16p.tile([P, 2, MT], F16, tag="fp", name="o2")
                    for g, bcol in (("i", 0), ("o", 4), ("u", 8)):
                        w = w8[g]
                        pj = [ps2.tile([P, MT], F32, tag=g,
                                       name=f"ps_{g}{j}") for j in (0, 1)]
                        for kd in range(2):
                            for j in (0, 1):
                                c0 = xc0 + j * MT
                                nc.tensor.matmul(
                                    pj[j][:], w[:, kd, :, ms],
                                    x8_s[:, kd, :, c0:c0 + MT],
                                    start=(kd == 0), stop=(kd == 1),
                                    perf_mode=DRMODE)
                        for j in (0, 1):
                            if g == "o":
                                g16 = o2[:, j]
                            else:
                                g16 = g16_tile(g, MT) if j == 0 else \
                                    g16_tile(g + "b", MT)
                            fn = TANH if g == "u" else SIG
                            nc.scalar.activation(
                                g16[:], pj[j][:], fn, scale=DQ,
                                bias=bias_s[:, bcol + mo:bcol + mo + 1])
                            w8[f"_{g}{j}"] = g16
                    oc0 = xc0 % 1024
                    for j in (0, 1):
                        i16 = w8[f"_i{j}"]
                        u16 = w8[f"_u{j}"]
                        c_sl = out_c[:, mo, oc0 + j * MT:oc0 + (j + 1) * MT]
                        nc.vector.tensor_mul(c_sl, i16[:], u16[:])
                    tt2 = g16p.tile([P, 2, MT], F16, tag="tt2", name="tt2")
                    nc.scalar.activation(
                        tt2.rearrange("p a b -> p (a b)"),
                        out_c[:, mo, oc0:oc0 + 2 * MT], TANH)
                    nc.vector.scalar_tensor_tensor(
                        out_h8[:, mo, oc0:oc0 + 2 * MT],
                        tt2.rearrange("p a b -> p (a b)"), HS,
                        o2.rearrange("p a b -> p (a b)"), MULT, MULT)

            def big_internal(w8, xc0, m, ch_h8, ch_c, out_c, oc0,
                             out_h8=None, out_h=None):
                """Internal fp8 level tile (m<=512 cols).  Children at
                ch_h8/ch_c cols 0..2m.  h out: fp8 (out_h8) or f32r."""
                hs8 = hsump.tile([P, 4, MT], F8, tag="hs8",
                                 name="hs8")[:, :, :m]
                nc.vector.tensor_add(hs8[:], ch_h8[:, :, 0:2 * m:2],
                                     ch_h8[:, :, 1:2 * m:2])
                for mo in range(4):
                    ms = slice(mo * P, (mo + 1) * P)
                    ps = {}
                    for g in ("i", "o", "u"):
                        p = ps2.tile([P, MT], F32, tag=g,
                                     name=f"ps_{g}")[:, :m]
                        w = w8[g]
                        for kd in range(2):
                            nc.tensor.matmul(
                                p[:], w[:, kd, :, ms],
                                x8_s[:, kd, :, xc0:xc0 + m],
                                start=(kd == 0), stop=False,
                                perf_mode=DRMODE)
                        for kd in range(2):
                            nc.tensor.matmul(
                                p[:], w[:, 2 + kd, :, ms],
                                hs8[:, 2 * kd:2 * kd + 2],
                                start=False, stop=(kd == 1),
                                perf_mode=DRMODE)
                        ps[g] = p
                    pf = ps1.tile([P, 2, MT], F32, tag="fl",
                                  name="ps_f2")[:, :, :m]
                    pfl = pf[:, 0]
                    pfr = pf[:, 1]
                    for kd in range(2):
                        w = w8["fx"][:, kd, :, ms]
                        xsl = x8_s[:, kd, :, xc0:xc0 + m]
                        nc.tensor.matmul(pfl[:], w, xsl, start=(kd == 0),
                                         stop=False, perf_mode=DRMODE)
                        nc.tensor.matmul(pfr[:], w, xsl, start=(kd == 0),
                                         stop=False, perf_mode=DRMODE)
                    for kd in range(2):
                        w = w8["fh"][:, kd, :, ms]
                        nc.tensor.matmul(
                            pfl[:], w, ch_h8[:, 2 * kd:2 * kd + 2, 0:2 * m:2],
                            start=False, stop=(kd == 1), perf_mode=DRMODE)
                        nc.tensor.matmul(
                            pfr[:], w, ch_h8[:, 2 * kd:2 * kd + 2, 1:2 * m:2],
                            start=False, stop=(kd == 1), perf_mode=DRMODE)
                    i16 = g16_tile("i", m)
                    o16 = g16_tile("o", m)
                    u16 = g16_tile("u", m)
                    f16_pair = g16p.tile([P, 2, MT], F16, tag="fp",
                                         name="f16_pair")[:, :, :m]
                    fl16 = f16_pair[:, 0]
                    fr16 = f16_pair[:, 1]
                    nc.scalar.activation(i16[:], ps["i"][:], SIG, scale=DQ,
                                         bias=bias_s[:, mo:mo + 1])
                    nc.scalar.activation(o16[:], ps["o"][:], SIG, scale=DQ,
                                         bias=bias_s[:, 4 + mo:5 + mo])
                    nc.scalar.activation(u16[:], ps["u"][:], TANH, scale=DQ,
                                         bias=bias_s[:, 8 + mo:9 + mo])
                    nc.scalar.activation(f16_pair[:], pf[:], SIG, scale=DQ,
                                         bias=bias_s[:, 12 + mo:13 + mo])
                    c_sl = out_c[:, mo, oc0:oc0 + m]
                    nc.vector.tensor_mul(c_sl, i16[:], u16[:])
                    t1 = tmp_tile(m, "t1")
                    nc.vector.tensor_mul(t1[:], fl16,
                                         ch_c[:, mo, 0:2 * m:2])
                    nc.vector.tensor_add(c_sl, c_sl, t1[:])
                    t2 = tmp_tile(m, "t2")
                    nc.vector.tensor_mul(t2[:], fr16,
                                         ch_c[:, mo, 1:2 * m:2])
                    nc.vector.tensor_add(c_sl, c_sl, t2[:])
                    tt = g16_tile("tt", m)
                    nc.scalar.activation(tt[:], c_sl, TANH)
                    if out_h8 is not None:
                        nc.vector.scalar_tensor_tensor(
                            out_h8[:, mo, oc0:oc0 + m], tt[:], HS, o16[:],
                            MULT, MULT)
                    else:
                        nc.vector.tensor_mul(out_h[:, mo, 0:m], o16[:],
                                             tt[:])

            def precompute_gx(w16):
                """Batched fp16 x-parts (+bias) for small+top nodes."""
                for mo in range(4):
                    ms = slice(mo * P, (mo + 1) * P)
                    for gi, (g, wnm, bcol) in enumerate(
                            (("i", "i", 0), ("o", "o", 4),
                             ("u", "u", 8), ("f", "fx", 12))):
                        w_s = w16[wnm]
                        ps = ps2.tile([P, MT], F32,
                                      tag=("i", "o", "u")[gi % 3],
                                      name="ps_pre")[:, :NSM]
                        for ko in range(4):
                            nc.tensor.matmul(
                                ps[:], w_s[:, ko, ms], x16_s[:, ko],
                                start=(ko == 0), stop=(ko == 3))
                        nc.scalar.activation(
                            gx[g][:, mo], ps[:], IDENT,
                            bias=bias_s[:, bcol + mo:bcol + mo + 1])

            def small_level(w16, m, gxoff, ch_h, ch_c, out_h, out_c):
                """Levels m<=128: fp16 h-part matmuls; x-parts from gx.
                m=1 levels padded to 2."""
                m = max(m, 2)
                hs = h16p.tile([P, 4, P], F16, tag="hs16",
                               name="hs")[:, :, :m]
                nc.vector.tensor_add(hs[:], ch_h[:, :, 0:2 * m:2],
                                     ch_h[:, :, 1:2 * m:2])
                if ch_h.dtype == F16:
                    hch = ch_h[:, :, 0:2 * m]
                else:
                    hch_t = h16p.tile([P, 4, 2 * P], F16, tag="hch16",
                                      name="hch")[:, :, :2 * m]
                    nc.vector.tensor_copy(hch_t[:], ch_h[:, :, 0:2 * m])
                    hch = hch_t[:]
                ps_i = ps2.tile([P, 4, P], F32, tag="i",
                                name="ps_i")[:, :, :m]
                ps_u = ps2.tile([P, 4, P], F32, tag="u",
                                name="ps_u")[:, :, :m]
                ps_o = ps2.tile([P, 4, P], F32, tag="o",
                                name="ps_o")[:, :, :m]
                gsl = slice(gxoff, gxoff + m)
                f_sb = fip.tile([P, 4, 2 * P], F32, tag="fint",
                                name="f_sb")[:, :, :2 * m]
                for mo in range(4):
                    ms = slice(mo * P, (mo + 1) * P)
                    psf = ps1.tile([P, 2, MT], F32, tag="fl",
                                   name="ps_f")[:, mo % 2, :2 * m]
                    for ko in range(4):
                        nc.tensor.matmul(
                            psf[:], w16["fh"][:, ko, ms], hch[:, ko],
                            start=(ko == 0), stop=(ko == 3))
                    nc.vector.tensor_add(
                        f_sb[:, mo].rearrange("p (m two) -> p m two", two=2),
                        psf.rearrange("p (m two) -> p m two", two=2),
                        gx["f"][:, mo, gsl, None].to_broadcast((P, m, 2)))
                for mo in range(4):
                    ms = slice(mo * P, (mo + 1) * P)
                    for wnm, ps in (("ih", ps_i), ("oh", ps_o),
                                    ("uh", ps_u)):
                        w_s = w16[wnm]
                        for ko in range(4):
                            nc.tensor.matmul(
                                ps[:, mo], w_s[:, ko, ms], hs[:, ko],
                                start=(ko == 0), stop=(ko == 3))
                nc.scalar.activation(f_sb[:], f_sb[:], SIG)
                io = gt.tile([P, 2, MT], F32, tag="io",
                             name="g_io").rearrange(
                    "p t (ho m) -> p t ho m", ho=4)[:, :, :, :m]
                nc.vector.tensor_add(io[:, 0], ps_i[:], gx["i"][:, :, gsl])
                nc.vector.tensor_add(io[:, 1], ps_o[:], gx["o"][:, :, gsl])
                nc.scalar.activation(io[:], io[:], SIG)
                u_sb = gt.tile([P, MT], F32, tag="u",
                               name="g_u").rearrange(
                    "p (ho m) -> p ho m", ho=4)[:, :, :m]
                nc.vector.tensor_add(u_sb[:], ps_u[:], gx["u"][:, :, gsl])
                nc.scalar.activation(u_sb[:], u_sb[:], TANH)
                gates = {"i": io[:, 0], "o": io[:, 1], "u": u_sb}
                c_sl = out_c[:, :, 0:m]
                h_sl = out_h[:, :, 0:m]
                c_l = ch_c[:, :, 0:2 * m:2]
                c_r = ch_c[:, :, 1:2 * m:2]
                nc.vector.tensor_mul(c_sl, gates["i"], gates["u"][:])
                t1 = tmp_tile3(m, "t1")
                nc.vector.tensor_mul(t1[:], f_sb[:, :, 0::2], c_l)
                nc.vector.tensor_add(c_sl, c_sl, t1[:])
                t2 = tmp_tile3(m, "t2")
                nc.vector.tensor_mul(t2[:], f_sb[:, :, 1::2], c_r)
                nc.vector.tensor_add(c_sl, c_sl, t2[:])
                tt = tmp_tile3(m, "tt")
                nc.scalar.activation(tt[:], c_sl, TANH)
                nc.vector.tensor_mul(h_sl, gates["o"], tt[:])

            if repeat == 0:
                nc.sync.dma_start(
                    hc_out[:],
                    xt8.bitcast(F32)[0:2].rearrange(
                        "a kd i n -> a (kd i) n")[:, :, 0:P])
            _x_rest_loaded = [False]
            for _rep in range(repeat):
                if serialize and _rep > 0:
                    # 1-elem token per 512-col subtile: rep N's first MMs
                    # depend on rep N-1's final hA (true serial latency)
                    nc.vector.scalar_tensor_tensor(
                        x8_s[:, 0, 0, 0:NBIG:512], hA[:, 0, 0:8], 0.0,
                        x8_s[:, 0, 0, 0:NBIG:512], MULT,
                        mybir.AluOpType.add)
                w8 = load_w8()
                if not _x_rest_loaded[0]:
                    load_x(first=False)
                    _x_rest_loaded[0] = True
                w16 = load_w16()
                with nc.named_scope("L14h0"):
                    leaf_half(w8, 0, cA, h8A)
                with nc.named_scope("pre"):
                    precompute_gx(w16)
                with nc.named_scope("L13j0"):
                    big_internal(w8, XB_OFF[13], 512, h8A, cA, cB, 0,
                                 out_h8=h8B)
                with nc.named_scope("L14h1"):
                    leaf_half(w8, 1024, cA, h8A)
                with nc.named_scope("L13j1"):
                    big_internal(w8, XB_OFF[13] + 512, 512, h8A, cA, cB, 512,
                                 out_h8=h8B)
                with nc.named_scope("L12"):
                    big_internal(w8, XB_OFF[12], 512, h8B, cB, cA, 0,
                                 out_h8=h8A)
                with nc.named_scope("L11"):
                    big_internal(w8, XB_OFF[11], 256, h8A, cA, cA2, 0,
                                 out_h=hA)
                if stop_after == "L11":
                    hf11 = h16p.tile([P, 4, 1], F32, tag="hfin",
                                     name="hf11")
                    nc.vector.tensor_copy(hf11[:], hA[:, :, 0:1])
                    nc.sync.dma_start(
                        hc_out[0:1].rearrange("one ko p -> p ko one"),
                        hf11[:])
                    nc.sync.dma_start(
                        hc_out[1:2].rearrange("one ko p -> p ko one"),
                        cA2[:, :, 0:1])
                    continue
                # small levels 10..3: ping-pong (hA,cA2) <-> (hB,cB2)
                cur_h, cur_c = hA, cA2
                for lvl in SMALL_LVLS:
                    nxt_h = hB if cur_h is hA else hA
                    nxt_c = cB2 if cur_c is cA2 else cA2
                    with nc.named_scope(f"L{lvl}"):
                        small_level(w16, M_SM[lvl], XS_OFF[lvl],
                                    cur_h, cur_c, nxt_h, nxt_c)
                    cur_h, cur_c = nxt_h, nxt_c

                if stop_after == "fakegather":
                    with nc.named_scope("fakegather"):
                        for r in range(NCORES):
                            nc.sync.dma_start(h3g[:, :, r:r + 1],
                                              cur_h[:, :, 0:1])
                            nc.sync.dma_start(c3g[:, :, r:r + 1],
                                              cur_c[:, :, 0:1])
                    with nc.named_scope("L2f"):
                        small_level(w16, 4, XS_TOP + 3, h3g, c3g, hA, cA2)
                    with nc.named_scope("L1f"):
                        small_level(w16, 2, XS_TOP + 1, hA, cA2, hB, cB2)
                    with nc.named_scope("L0f"):
                        small_level(w16, 1, XS_TOP + 0, hB, cB2, hA, cA2)
                    hff = h16p.tile([P, 4, 1], F32, tag="hfin", name="hff")
                    nc.vector.tensor_copy(hff[:], hA[:, :, 0:1])
                    nc.sync.dma_start(
                        hc_out[0:1].rearrange("one ko p -> p ko one"),
                        hff[:])
                    nc.sync.dma_start(
                        hc_out[1:2].rearrange("one ko p -> p ko one"),
                        cA2[:, :, 0:1])
                    continue
                if stop_after == "L3":
                    hf3 = h16p.tile([P, 4, 1], F32, tag="hfin", name="hf3")
                    nc.vector.tensor_copy(hf3[:], cur_h[:, :, 0:1])
                    nc.sync.dma_start(
                        hc_out[0:1].rearrange("one ko p -> p ko one"),
                        hf3[:])
                    nc.sync.dma_start(
                        hc_out[1:2].rearrange("one ko p -> p ko one"),
                        cur_c[:, :, 0:1])
                    continue
                with nc.named_scope("gather"):
                    cc_in = dram.tile([2, 4, P], F32R, name="cc_in")
                    cc_out = dram.tile([8, 2, 4, P], F32R, name="cc_out")
                    h3f = h16p.tile([P, 4, 1], F32R, tag="h3f",
                                    name="h3f")
                    nc.vector.tensor_copy(h3f[:], cur_h[:, :, 0:1])
                    nc.sync.dma_start(
                        cc_in[0:1].rearrange("one ko p -> p ko one"),
                        h3f[:])
                    nc.sync.dma_start(
                        cc_in[1:2].rearrange("one ko p -> p ko one"),
                        cur_c.bitcast(F32R)[:, :, 0:1])
                    if sim1:
                        for r in range(NCORES):
                            nc.sync.dma_start(cc_out[r], cc_in[:])
                    else:
                        nc.gpsimd.collective_compute(
                            "AllGather", mybir.AluOpType.bypass,
                            replica_groups=[list(range(NCORES))],
                            ins=[cc_in.opt()], outs=[cc_out.opt()])
                    for ko in range(4):
                        nc.sync.dma_start(
                            h3g[:, ko, 0:8],
                            cc_out[:, 0, ko].rearrange("r p -> p r"))
                        nc.sync.dma_start(
                            c3g[:, ko, 0:8],
                            cc_out.bitcast(F32)[:, 1, ko].rearrange(
                                "r p -> p r"))

                with nc.named_scope("L2"):
                    small_level(w16, 4, XS_TOP + 3, h3g, c3g, hA, cA2)
                with nc.named_scope("L1"):
                    small_level(w16, 2, XS_TOP + 1, hA, cA2, hB, cB2)
                with nc.named_scope("L0"):
                    small_level(w16, 1, XS_TOP + 0, hB, cB2, hA, cA2)

                hfin = h16p.tile([P, 4, 1], F32, tag="hfin", name="hfin")
                nc.vector.tensor_copy(hfin[:], hA[:, :, 0:1])
                nc.sync.dma_start(
                    hc_out[0:1].rearrange("one ko p -> p ko one"),
                    hfin[:])
                nc.sync.dma_start(
                    hc_out[1:2].rearrange("one ko p -> p ko one"),
                    cA2[:, :, 0:1])

    nc.compile()
    return nc


def _prep_inputs(x, Wi, bi, Wf, bf, Wo, bo, Wu, bu):
    import ml_dtypes
    E4 = ml_dtypes.float8_e4m3
    Wi, Wf, Wo, Wu = (np.asarray(w, np.float32) for w in (Wi, Wf, Wo, Wu))

    def wt8(wpart):  # [H(M), 512(K)] -> [P, 2(kd), 2, H] fp8 scaled
        a = wpart.T.reshape(2, 2, P, H).transpose(2, 0, 1, 3)
        return np.ascontiguousarray(a * WS).astype(E4)

    def wt16(wpart):  # [H, 512] -> [P, 4(ko), H] fp16
        a = wpart.T.reshape(4, P, H).transpose(1, 0, 2)
        return np.ascontiguousarray(a).astype(np.float16)

    w8 = {}
    for nm, w in (("i", Wi), ("o", Wo), ("u", Wu)):
        w8[nm] = np.concatenate([wt8(w[:, :D]), wt8(w[:, D:])], axis=1)
    w8fx, w8fh = wt8(Wf[:, :D]), wt8(Wf[:, D:])
    w16 = {nm: wt16(w[:, :D]) for nm, w in
           (("i", Wi), ("o", Wo), ("u", Wu), ("fx", Wf))}
    w16h = {nm: wt16(w[:, D:]) for nm, w in
            (("ih", Wi), ("oh", Wo), ("uh", Wu), ("fh", Wf))}
    bias = np.stack(
        [np.asarray(b, np.float32).reshape(4, P) for b in (bi, bo, bu, bf)],
        axis=0)
    bias = np.ascontiguousarray(bias.reshape(16, P).T).astype(np.float32)

    x = np.asarray(x, dtype=np.float32)
    in_maps = []
    for j in range(NCORES):
        bcols = []
        for lvl in BIG_LVLS:
            s, m = 2 ** lvl - 1, M_BIG[lvl]
            bcols.extend(range(s + j * m, s + (j + 1) * m))
        scols = []
        for lvl in SMALL_LVLS:
            s, m = 2 ** lvl - 1, M_SM[lvl]
            scols.extend(range(s + j * m, s + (j + 1) * m))
        scols.extend(range(7))
        xb = x[bcols]                            # [NBIG, 512]
        x8 = np.ascontiguousarray(
            xb.T.reshape(2, 2, P, NBIG).transpose(2, 0, 1, 3) * XS
        ).astype(E4)
        xsm = x[scols]                           # [NSM, 512]
        x16 = np.ascontiguousarray(
            xsm.T.reshape(4, P, NSM).transpose(1, 0, 2)).astype(np.float16)
        in_maps.append({
            "xt8": x8, "xt16": x16,
            "w8i": w8["i"], "w8o": w8["o"], "w8u": w8["u"],
            "w8fx": w8fx, "w8fh": w8fh,
            "w16i": w16["i"], "w16o": w16["o"], "w16u": w16["u"],
            "w16fx": w16["fx"],
            "wih": w16h["ih"], "woh": w16h["oh"], "wuh": w16h["uh"],
            "wfh16": w16h["fh"], "bias": bias,
        })
    return in_maps


def _make_runner(nc, n_cores=NCORES):
    """Build the sharded jitted callable once (mirrors
    bass2jax.run_bass_via_pjrt) so repeated timed calls don't recompile."""
    import jax
    from jax.sharding import Mesh, PartitionSpec
    from jax.experimental.shard_map import shard_map
    from concourse import bass2jax
    from concourse.bass2jax import _bass_exec_p, install_neuronx_cc_hook

    install_neuronx_cc_hook()
    partition_name = (nc.partition_id_tensor.name
                      if nc.partition_id_tensor else None)
    in_names, out_names, out_avals, zero_outs = [], [], [], []
    for alloc in nc.m.functions[0].allocations:
        if not isinstance(alloc, mybir.MemoryLocationSet):
            continue
        name = alloc.memorylocations[0].name
        if alloc.kind == "ExternalInput":
            if name != partition_name:
                in_names.append(name)
        elif alloc.kind == "ExternalOutput":
            shape = tuple(alloc.tensor_shape)
            dtype = mybir.dt.np(alloc.dtype)
            out_names.append(name)
            out_avals.append(jax.core.ShapedArray(shape, dtype))
            zero_outs.append(np.zeros(shape, dtype))
    n_params = len(in_names)
    n_outs = len(out_avals)
    full_in_names = list(in_names) + list(out_names)
    if partition_name is not None:
        full_in_names.append(partition_name)

    def _body(*args):
        operands = list(args)
        if partition_name is not None:
            operands.append(bass2jax.partition_id_tensor())
        outs = _bass_exec_p.bind(
            *operands,
            out_avals=tuple(out_avals),
            in_names=tuple(full_in_names),
            out_names=tuple(out_names),
            lowering_input_output_aliases=(),
            sim_require_finite=True,
            sim_require_nnan=True,
            nc=nc,
        )
        return tuple(outs)

    devices = jax.devices()[:n_cores]
    mesh = Mesh(np.asarray(devices), ("core",))
    in_specs = (PartitionSpec("core"),) * (n_params + n_outs)
    out_specs = (PartitionSpec("core"),) * n_outs
    donate = tuple(range(n_params, n_params + n_outs))
    sharded = jax.jit(
        shard_map(_body, mesh=mesh, in_specs=in_specs,
                  out_specs=out_specs, check_rep=False),
        donate_argnums=donate, keep_unused=True)

    def run(in_maps):
        per_core = [[np.asarray(m[name]) for name in in_names]
                    for m in in_maps]
        concat_in = [np.concatenate([per_core[c][i] for c in range(n_cores)],
                                    axis=0) for i in range(n_params)]
        concat_zeros = [np.zeros((n_cores * z.shape[0], *z.shape[1:]),
                                 z.dtype) for z in zero_outs]
        outs = sharded(*concat_in, *concat_zeros)
        jax.block_until_ready(outs)
        return outs

    return run, out_avals


def _make_caller(nc):
    """Non-blocking sharded caller for bench (dummy-input builds)."""
    import jax
    from jax.sharding import Mesh, PartitionSpec
    from jax.experimental.shard_map import shard_map
    from concourse import bass2jax
    from concourse.bass2jax import _bass_exec_p, install_neuronx_cc_hook

    install_neuronx_cc_hook()
    partition_name = (nc.partition_id_tensor.name
                      if nc.partition_id_tensor else None)
    out_names, out_avals, zero_outs = [], [], []
    for alloc in nc.m.functions[0].allocations:
        if not isinstance(alloc, mybir.MemoryLocationSet):
            continue
        if alloc.kind == "ExternalOutput":
            shape = tuple(alloc.tensor_shape)
            dtype = mybir.dt.np(alloc.dtype)
            out_names.append(alloc.memorylocations[0].name)
            out_avals.append(jax.core.ShapedArray(shape, dtype))
            zero_outs.append(np.zeros(shape, dtype))
    full_in_names = list(out_names)
    if partition_name is not None:
        full_in_names.append(partition_name)

    def _body(*args):
        operands = list(args)
        if partition_name is not None:
            operands.append(bass2jax.partition_id_tensor())
        return tuple(_bass_exec_p.bind(
            *operands, out_avals=tuple(out_avals),
            in_names=tuple(full_in_names), out_names=tuple(out_names),
            lowering_input_output_aliases=(), sim_require_finite=True,
            sim_require_nnan=True, nc=nc))

    devices = jax.devices()[:NCORES]
    mesh = Mesh(np.asarray(devices), ("core",))
    n_outs = len(out_avals)
    sharded = jax.jit(
        shard_map(_body, mesh=mesh,
                  in_specs=(PartitionSpec("core"),) * n_outs,
                  out_specs=(PartitionSpec("core"),) * n_outs,
                  check_rep=False),
        donate_argnums=tuple(range(n_outs)), keep_unused=True)

    def call():
        czeros = [np.zeros((NCORES * z.shape[0], *z.shape[1:]), z.dtype)
                  for z in zero_outs]
        return sharded(*czeros)
    return call


def bench(reps=(2, 18), iters=40, stop_after=None, serialize=True,
          batches=8):
    """Async-pipelined, batch-interleaved delta timing."""
    import time
    import jax
    calls = []
    for rep in reps:
        nc = _build_nc(repeat=rep, bench_dummy=True,
                       stop_after=stop_after, serialize=serialize)
        call = _make_caller(nc)
        jax.block_until_ready(call())
        calls.append(call)

    def batch(call):
        t0 = time.perf_counter()
        outs = [call() for _ in range(iters)]
        jax.block_until_ready(outs)
        return (time.perf_counter() - t0) / iters

    batch(calls[0]); batch(calls[1])  # extra warm
    diffs = []
    for k in range(batches):
        a, b = (0, 1) if k % 2 == 0 else (1, 0)
        ta = batch(calls[a])
        tb = batch(calls[b])
        d = (tb - ta) if a == 0 else (ta - tb)
        diffs.append(d)
    diffs.sort()
    n = len(diffs)
    mid = diffs[n // 4: n - n // 4] or diffs
    med = sum(mid) / len(mid)
    print(f"  bench diffs(ms): {[f'{d*1e3:.2f}' for d in diffs]}")
    return med / (reps[1] - reps[0]) * 1e9


def kernel(x, Wi, bi, Wf, bf, Wo, bo, Wu, bu):
    if "nc" not in _CACHE:
        _CACHE["nc"] = _build_nc()
    nc = _CACHE["nc"]
    in_maps = _prep_inputs(x, Wi, bi, Wf, bf, Wo, bo, Wu, bu)
    res = run_bass_kernel_spmd(nc, in_maps, core_ids=list(range(NCORES)))
    out = res.results[0]["hc_out"]               # [2, 4, 128]
    h0 = np.ascontiguousarray(out[0].reshape(H)).astype(np.float32)
    c0 = np.ascontiguousarray(out[1].reshape(H)).astype(np.float32)
    return h0, c0
